# revision 13
# baseline (speedup 1.0000x reference)
"""Causal GQA multi-head attention (RMSNorm-QK + RoPE) on 8 Trainium2 cores.

Sharding: (batch, kv-group). Core c owns batch c//4 and GQA group c%4,
i.e. 4 q heads + 1 kv head for one batch of 2048 tokens. This splits the
total work exactly 8 ways with zero duplicated projection flops (the old
head-sharding computed each kv head twice and projected both batches on
every core). Each core emits a partial [S, D] output (row-sharded Wo);
the host sums the 4 partials per batch.

Per-core structure (all matmuls bf16, K=M=128, N<=512):
  - proj: x[D, S] @ wqkv -> qT (4 heads), kT, vT in [dh, token] layout.
    First block is emitted k-outer so the PE starts after ~2 DMA tiles.
  - rmsnorm: sumsq via ones-matmul; rsqrt as exp(-0.5*ln(x)) so the
    whole kernel uses ONE activation table set (natural_log_exp);
    q-side rs (absorbs 1/sqrt(dh)) and k-side rs are folded into the
    qT/kT tiles themselves during the RoPE epilogue, so attention exp
    needs no per-partition scale.
  - attention: scoresT blocks [keys, queries]; causal mask folded into
    the scores psum via a tiny [128,128] identity-matmul add of -30000
    on the diagonal block only; exp WITHOUT max-subtraction; rowsums
    via accumulated ones-matmul; the inner loop is software-pipelined
    (scores run LAG=2 chunks ahead of AV) so the PE never waits on the
    scalar engine's exp.
  - Wo: row-sharded partial, interleaved per query-block.
"""

import sys

sys.path.insert(0, "/opt/trn_rl_repo")

from contextlib import ExitStack

import ml_dtypes
import numpy as np

import concourse.bass as bass
import concourse.tile as tile
from concourse import bacc, mybir
from concourse.bass_utils import run_bass_kernel_spmd
from concourse.masks import make_identity

B, S, D = 2, 2048, 2048
H, HKV, DH = 16, 4, 128
P = 128
NCORES = 8
HPC = 4  # q heads per core
EPS = 1e-6
ROPE_THETA = 10000.0
BF = mybir.dt.bfloat16
F32 = mybir.dt.float32
BFNP = ml_dtypes.bfloat16

Copy = mybir.ActivationFunctionType.Copy
Exp = mybir.ActivationFunctionType.Exp
Sqrt = mybir.ActivationFunctionType.Sqrt
MULT = mybir.AluOpType.mult
ADD = mybir.AluOpType.add
SUB = mybir.AluOpType.subtract

NBLK = 4  # 512-token blocks
BLK = S // NBLK


def _body(ctx: ExitStack, tc: tile.TileContext, xt, wqkv, wo, cos, sins, masktri, gq, gk, out):
    nc = tc.nc

    const = ctx.enter_context(tc.tile_pool(name="const", bufs=1))
    res = ctx.enter_context(tc.tile_pool(name="res", bufs=1))
    xt_pool = ctx.enter_context(tc.tile_pool(name="xtp", bufs=2))
    sq_pool = ctx.enter_context(tc.tile_pool(name="sqp", bufs=3))
    row_pool = ctx.enter_context(tc.tile_pool(name="row", bufs=6))
    row2_pool = ctx.enter_context(tc.tile_pool(name="row2", bufs=3))
    rope_pool = ctx.enter_context(tc.tile_pool(name="rop", bufs=2))
    exp_pool = ctx.enter_context(tc.tile_pool(name="exq", bufs=6))
    attu_pool = ctx.enter_context(tc.tile_pool(name="attu", bufs=2))
    att_pool = ctx.enter_context(tc.tile_pool(name="attp", bufs=2))
    osb_pool = ctx.enter_context(tc.tile_pool(name="osb", bufs=2))
    # PSUM: 8 banks = sc(3) + attps(1) + sumps(1) + pp(3)
    scp = ctx.enter_context(tc.tile_pool(name="scp", bufs=3, space="PSUM"))
    attps = ctx.enter_context(tc.tile_pool(name="atps", bufs=1, space="PSUM"))
    sumps = ctx.enter_context(tc.tile_pool(name="smps", bufs=1, space="PSUM"))
    pp = ctx.enter_context(tc.tile_pool(name="pp", bufs=3, space="PSUM"))

    # ---- constants / resident weights ----
    ones_bf = const.tile([P, 1], BF, name="ones", tag="ones")
    nc.vector.memset(ones_bf[:], 1.0)
    ones1 = const.tile([1, P], BF, name="ones1", tag="ones1")
    nc.vector.memset(ones1[:], 1.0)
    ident = const.tile([P, P], BF, name="ident", tag="ident")
    make_identity(nc, ident[:])
    gq_t = const.tile([P, 1], F32, name="gq", tag="gq")
    gk_t = const.tile([P, 1], F32, name="gk", tag="gk")
    cos_t = const.tile([P, S], BF, name="cos", tag="cos")
    sins_t = const.tile([P, S], BF, name="sins", tag="sins")
    mask_t = const.tile([P, P], BF, name="mask", tag="mask")
    epsq_t = const.tile([1, 1], F32, name="epsq", tag="epsq")
    nc.vector.memset(epsq_t[:], P * EPS)
    epsk_t = const.tile([1, 1], F32, name="epsk", tag="epsk")
    nc.vector.memset(epsk_t[:], EPS)
    wqkv_sb = [const.tile([P, 768], BF, name=f"wqkv{k}", tag=f"wqkv{k}") for k in range(16)]
    wo_sb = [const.tile([P, D], BF, name=f"wo{h}", tag=f"wo{h}") for h in range(HPC)]

    # resident activations, [dh, token] layouts
    qT = [res.tile([P, S], BF, name=f"qT{h}", tag=f"qT{h}") for h in range(HPC)]
    kT = res.tile([P, S], BF, name="kT", tag="kT")
    vT = res.tile([P, S], BF, name="vT", tag="vT")
    v_kd = res.tile([P, S], BF, name="vkd", tag="vkd")  # [keys, dh] chunks

    xts = [[None] * 16 for _ in range(NBLK)]

    def dma_block(nb):
        for k in range(16):
            t = xt_pool.tile([P, BLK], BF, name=f"xt{k}", tag=f"xt{k}")
            nc.sync.dma_start(t[:], xt[k, :, nb * BLK:(nb + 1) * BLK])
            xts[nb][k] = t

    # preamble DMAs: tiny scales, then wqkv/x-block-0 interleaved so the
    # first matmul can start after ~2 tiles have landed.
    nc.sync.dma_start(gq_t[:], gq[:])
    nc.sync.dma_start(gk_t[:], gk[:])
    for k in range(16):
        nc.sync.dma_start(wqkv_sb[k][:], wqkv[k])
        t = xt_pool.tile([P, BLK], BF, name=f"xt{k}", tag=f"xt{k}")
        nc.sync.dma_start(t[:], xt[k, :, 0:BLK])
        xts[0][k] = t
    nc.sync.dma_start(cos_t[:], cos[:])
    nc.sync.dma_start(sins_t[:], sins[:])
    nc.sync.dma_start(mask_t[:], masktri[:])

    def bcast_row(row_f32):
        """Broadcast a [1,N] f32 row to a [128,N] f32 PSUM tile via two
        accumulating K=1 ones-matmuls over a bf16 hi/lo split (~2^-16 exact)."""
        hi = row2_pool.tile([1, BLK], BF, name="hi", tag="hi")
        lo_f = row_pool.tile([1, BLK], F32, name="lof", tag="row")
        lo = row2_pool.tile([1, BLK], BF, name="lo", tag="lo")
        nc.vector.tensor_copy(hi[:], row_f32[:])
        nc.vector.tensor_tensor(lo_f[:], row_f32[:], hi[:], SUB)
        nc.vector.tensor_copy(lo[:], lo_f[:])
        rsb = scp.tile([P, BLK], F32, name="rsb", tag="sc")
        nc.tensor.matmul(rsb[:], ones1[:], hi[:], start=True, stop=False,
                         skip_group_check=True)
        nc.tensor.matmul(rsb[:], ones1[:], lo[:], start=False, stop=True,
                         skip_group_check=True)
        return rsb

    # ---- phase 1: fused qkv projection + rmsnorm/rope epilogue ----
    def rope_tile(dst, cols, rsb):
        """dst = (dst*cos + rot(dst)*sin) * rsb, in place; sins has the
        sign of the rotation baked into its first 64 rows."""
        t1 = rope_pool.tile([P, BLK], BF, name="t1", tag="t1")
        t2 = rope_pool.tile([P, BLK], BF, name="t2", tag="t2")
        nc.vector.tensor_copy(t2[0:64, :], dst[64:128, :])
        nc.vector.tensor_copy(t2[64:128, :], dst[0:64, :])
        nc.vector.tensor_tensor(t2[:], t2[:], sins_t[:, cols], MULT)
        nc.vector.tensor_tensor(t1[:], dst[:], cos_t[:, cols], MULT)
        nc.vector.tensor_tensor(t1[:], t1[:], t2[:], ADD)
        nc.vector.tensor_tensor(dst[:], t1[:], rsb[:], MULT)

    def epilogue(nb, m, ps):
        cols = slice(nb * BLK, (nb + 1) * BLK)
        if m == 5:  # v: evict + transpose to [keys, dh] chunks
            nc.vector.tensor_copy(vT[:, cols], ps[:])
            pst = pp.tile([P, BLK], BF, name="pst", tag="pp")
            for i in range(4):
                c = nb * 4 + i
                nc.tensor.transpose(pst[:, i * P:(i + 1) * P], vT[:, c * P:(c + 1) * P], ident[:])
            nc.scalar.copy(v_kd[:, cols], pst[:])
            return
        if m < 4:
            dst = qT[m]
            nc.scalar.activation(dst[:, cols], ps[:], Copy, bias=0.0, scale=gq_t[:])
        else:
            dst = kT
            nc.scalar.activation(dst[:, cols], ps[:], Copy, bias=0.0, scale=gk_t[:])
        sq = sq_pool.tile([P, BLK], BF, name="sq", tag="sq")
        nc.vector.tensor_tensor(sq[:], dst[:, cols], dst[:, cols], MULT)
        psr = pp.tile([P, BLK], F32, name="psr", tag="pp")
        nc.tensor.matmul(psr[:1, :], ones_bf[:], sq[:], start=True, stop=True,
                         skip_group_check=True)
        row = row_pool.tile([1, BLK], F32, name="row", tag="row")
        if m < 4:
            # rs_q = 1/sqrt(sumsq + 128*eps) == rsqrt(var+eps)/sqrt(dh)
            nc.scalar.activation(row[:], psr[:1, :], Sqrt, bias=epsq_t[:1, :])
        else:
            # rs_k = rsqrt(var + eps)
            nc.scalar.activation(row[:], psr[:1, :], Sqrt, bias=epsk_t[:1, :], scale=1.0 / P)
        rrow = row_pool.tile([1, BLK], F32, name="rrow", tag="row")
        nc.vector.reciprocal_approx_fast(rrow[:], row[:])
        rope_tile(dst[:, cols], cols, bcast_row(rrow))

    def proj_block(nb):
        if nb + 1 < NBLK:
            dma_block(nb + 1)
        if nb == 0:
            # k-outer: DMA-paced warmup; uses 6 psum banks across pools
            psms = [scp.tile([P, BLK], F32, name="ps", tag="sc") for _ in range(3)]
            psms.append(attps.tile([P, BLK], F32, name="ps", tag="attps"))
            psms.append(sumps.tile([P, BLK], F32, name="ps", tag="sumps"))
            psms.append(pp.tile([P, BLK], F32, name="ps", tag="pp"))
            for k in range(16):
                for m in range(6):
                    nc.tensor.matmul(
                        psms[m][:], wqkv_sb[k][:, m * P:(m + 1) * P], xts[0][k][:],
                        start=(k == 0), stop=(k == 15), skip_group_check=True,
                    )
            epilogue(0, 5, psms[5])  # v first: frees its pp slot early
            for m in range(5):
                epilogue(0, m, psms[m])
        else:
            for m in range(6):
                ps = pp.tile([P, BLK], F32, name="ps", tag="pp")
                for k in range(16):
                    nc.tensor.matmul(
                        ps[:], wqkv_sb[k][:, m * P:(m + 1) * P], xts[nb][k][:],
                        start=(k == 0), stop=(k == 15), skip_group_check=True,
                    )
                epilogue(nb, m, ps)
        if nb == 1:
            for h in range(HPC):
                nc.sync.dma_start(wo_sb[h][:], wo[h])

    # ---- phase 2: attention (software-pipelined) + Wo per query block ----
    def attn_head(h, qt):
        nkc = 4 * qt + 4
        q0 = qt * BLK
        ps_att = attps.tile([P, BLK], F32, name="psA", tag="attps")
        ps_sum = sumps.tile([P, BLK], F32, name="psB", tag="sumps")

        def scores(kc):
            off = max(0, P * kc - q0)
            ps_s = scp.tile([P, BLK], F32, name="psS", tag="sc")
            nc.tensor.matmul(
                ps_s[:, off:], kT[:, kc * P:(kc + 1) * P], qT[h][:, q0 + off:q0 + BLK],
                start=True, stop=(kc < 4 * qt), skip_group_check=True,
            )
            if kc >= 4 * qt:  # diagonal block: add -30000 upper triangle
                nc.tensor.matmul(
                    ps_s[:, off:off + P], ident[:], mask_t[:],
                    start=False, stop=True, skip_group_check=True,
                )
            ex = exp_pool.tile([P, BLK], BF, name="ex", tag="ex")
            nc.scalar.activation(ex[:, off:], ps_s[:, off:], Exp)
            return kc, off, ex

        def av(kc, off, ex):
            nc.tensor.matmul(
                ps_att[:, off:], v_kd[:, kc * P:(kc + 1) * P], ex[:, off:],
                start=(kc == 0), stop=(kc == nkc - 1), skip_group_check=True,
            )
            nc.tensor.matmul(
                ps_sum[:1, off:], ones_bf[:], ex[:, off:],
                start=(kc == 0), stop=(kc == nkc - 1), skip_group_check=True,
            )

        LAG = 2
        pend = []
        for kc in range(nkc):
            pend.append(scores(kc))
            if len(pend) > LAG:
                av(*pend.pop(0))
        while pend:
            av(*pend.pop(0))

        rrow = row_pool.tile([1, BLK], F32, name="rrow", tag="row")
        nc.vector.reciprocal_approx_fast(rrow[:], ps_sum[:1, :])
        rsb = bcast_row(rrow)
        att_un = attu_pool.tile([P, BLK], BF, name="attu", tag="attu")
        nc.vector.tensor_copy(att_un[:], ps_att[:])
        a = att_pool.tile([P, BLK], BF, name=f"att{h}", tag=f"att{h}")
        nc.vector.tensor_tensor(a[:], att_un[:], rsb[:], MULT)
        return a

    def wo_block(qt, atts):
        q0 = qt * BLK
        for tc4 in range(4):
            osb = osb_pool.tile([P, D], BF, name="osb", tag="osb")
            for et in range(4):
                ps = pp.tile([P, 512], F32, name="pso", tag="pp")
                for h2 in range(HPC):
                    nc.tensor.matmul(
                        ps[:], atts[h2][:, tc4 * P:(tc4 + 1) * P],
                        wo_sb[h2][:, et * 512:(et + 1) * 512],
                        start=(h2 == 0), stop=(h2 == HPC - 1), skip_group_check=True,
                    )
                nc.vector.tensor_copy(osb[:, et * 512:(et + 1) * 512], ps[:])
            nc.sync.dma_start(out[q0 + tc4 * P:q0 + (tc4 + 1) * P, :], osb[:])

    for nb in range(NBLK):
        proj_block(nb)
    for qt in range(NBLK):
        atts = [attn_head(h, qt) for h in range(HPC)]
        wo_block(qt, atts)


_NC_CACHE = None


def build_nc():
    global _NC_CACHE
    if _NC_CACHE is not None:
        return _NC_CACHE
    nc = bacc.Bacc(None, target_bir_lowering=False)
    xt = nc.dram_tensor("xt", [16, P, S], BF, kind="ExternalInput")
    wqkv = nc.dram_tensor("wqkv", [16, P, 768], BF, kind="ExternalInput")
    wo = nc.dram_tensor("wo", [HPC, P, D], BF, kind="ExternalInput")
    cos = nc.dram_tensor("cos", [P, S], BF, kind="ExternalInput")
    sins = nc.dram_tensor("sins", [P, S], BF, kind="ExternalInput")
    masktri = nc.dram_tensor("masktri", [P, P], BF, kind="ExternalInput")
    gq = nc.dram_tensor("gq", [P, 1], F32, kind="ExternalInput")
    gk = nc.dram_tensor("gk", [P, 1], F32, kind="ExternalInput")
    out = nc.dram_tensor("out", [S, D], BF, kind="ExternalOutput")
    with tile.TileContext(nc) as tc:
        with ExitStack() as ctx:
            _body(ctx, tc, xt[:], wqkv[:], wo[:], cos[:], sins[:], masktri[:],
                  gq[:], gk[:], out[:])
    nc.compile()
    _NC_CACHE = nc
    return nc


def _host_tables():
    pos = np.arange(S, dtype=np.float64)
    inv_freq = 1.0 / (ROPE_THETA ** (np.arange(0, DH, 2, dtype=np.float64) / DH))
    ang = pos[:, None] * inv_freq[None, :]  # [S, 64]
    cos_s = np.concatenate([np.cos(ang), np.cos(ang)], axis=-1)  # [S, 128]
    sin_s = np.concatenate([np.sin(ang), np.sin(ang)], axis=-1)
    cos_full = np.ascontiguousarray(cos_s.T).astype(BFNP)  # [128, S]
    sins = sin_s.T.copy()
    sins[0:64] *= -1.0  # rotation sign baked in
    sins = np.ascontiguousarray(sins).astype(BFNP)
    j = np.arange(P)[:, None]
    i = np.arange(P)[None, :]
    masktri = np.where(j <= i, 0.0, -30000.0).astype(BFNP)  # [keys, queries]
    return cos_full, sins, masktri


def kernel(qkv, Wq, Wk, Wv, Wo, q_gamma, k_gamma):
    qkv = np.asarray(qkv, dtype=np.float32)
    Wq = np.asarray(Wq, dtype=np.float32)
    Wk = np.asarray(Wk, dtype=np.float32)
    Wv = np.asarray(Wv, dtype=np.float32)
    Wo = np.asarray(Wo, dtype=np.float32)
    q_gamma = np.asarray(q_gamma, dtype=np.float32)
    k_gamma = np.asarray(k_gamma, dtype=np.float32)

    nc = build_nc()
    cos_full, sins, masktri = _host_tables()
    gq = np.ascontiguousarray(q_gamma.reshape(P, 1))
    gk = np.ascontiguousarray(k_gamma.reshape(P, 1))
    xts = [np.ascontiguousarray(qkv[b].T).astype(BFNP).reshape(16, P, S) for b in range(B)]

    in_maps = []
    for c in range(NCORES):
        b, g = c // 4, c % 4
        wq_c = Wq[4 * g * DH:(4 * g + 4) * DH, :]  # [512, D]
        wk_c = Wk[g * DH:(g + 1) * DH, :]  # [128, D]
        wv_c = Wv[g * DH:(g + 1) * DH, :]
        wqkv_c = np.concatenate([wq_c, wk_c, wv_c], axis=0).T  # [D, 768]
        wqkv_c = np.ascontiguousarray(wqkv_c).astype(BFNP).reshape(16, P, 768)
        wo_c = np.stack(
            [np.ascontiguousarray(Wo[:, (4 * g + h) * DH:(4 * g + h + 1) * DH].T)
             for h in range(HPC)]
        ).astype(BFNP)  # [4, 128, D]
        in_maps.append({
            "xt": xts[b], "wqkv": wqkv_c, "wo": wo_c,
            "cos": cos_full, "sins": sins, "masktri": masktri,
            "gq": gq, "gk": gk,
        })

    res = run_bass_kernel_spmd(nc, in_maps, core_ids=list(range(NCORES)))
    full = np.empty((B, S, D), np.float32)
    for b in range(B):
        acc = res.results[4 * b]["out"].astype(np.float32)
        for g in range(1, 4):
            acc += res.results[4 * b + g]["out"].astype(np.float32)
        full[b] = acc
    return full


# revision 19
# speedup vs baseline: 1.2019x; 1.2019x over previous
"""Causal GQA multi-head attention (RMSNorm-QK + RoPE) on 8 Trainium2 cores.

Sharding: (batch, kv-group). Core c owns batch c//4 and GQA group c%4,
i.e. 4 q heads + 1 kv head for one batch of 2048 tokens. This splits the
total work exactly 8 ways with zero duplicated projection flops (the old
head-sharding computed each kv head twice and projected both batches on
every core). Each core emits a partial [S, D] output (row-sharded Wo);
the host sums the 4 partials per batch.

Per-core structure (all matmuls bf16, K=M=128, N<=512):
  - proj: x[D, S] @ wqkv -> qT (4 heads), kT, vT in [dh, token] layout.
    First block is emitted k-outer so the PE starts after ~2 DMA tiles.
  - rmsnorm: sumsq via ones-matmul; rsqrt as exp(-0.5*ln(x)) so the
    whole kernel uses ONE activation table set (natural_log_exp);
    q-side rs (absorbs 1/sqrt(dh)) and k-side rs are folded into the
    qT/kT tiles themselves during the RoPE epilogue, so attention exp
    needs no per-partition scale.
  - attention: scoresT blocks [keys, queries]; causal mask folded into
    the scores psum via a tiny [128,128] identity-matmul add of -30000
    on the diagonal block only; exp WITHOUT max-subtraction; rowsums
    via accumulated ones-matmul; the inner loop is software-pipelined
    (scores run LAG=2 chunks ahead of AV) so the PE never waits on the
    scalar engine's exp.
  - Wo: row-sharded partial, interleaved per query-block.
"""

import sys

sys.path.insert(0, "/opt/trn_rl_repo")

from contextlib import ExitStack

import ml_dtypes
import numpy as np

import concourse.bass as bass
import concourse.tile as tile
from concourse import bacc, mybir
from concourse.bass_utils import run_bass_kernel_spmd
from concourse.masks import make_identity

B, S, D = 2, 2048, 2048
H, HKV, DH = 16, 4, 128
P = 128
NCORES = 8
HPC = 4  # q heads per core
EPS = 1e-6
ROPE_THETA = 10000.0
BF = mybir.dt.bfloat16
F32 = mybir.dt.float32
BFNP = ml_dtypes.bfloat16

Copy = mybir.ActivationFunctionType.Copy
Exp = mybir.ActivationFunctionType.Exp
Sqrt = mybir.ActivationFunctionType.Sqrt
MULT = mybir.AluOpType.mult
ADD = mybir.AluOpType.add
SUB = mybir.AluOpType.subtract

NBLK = 4  # 512-token blocks
BLK = S // NBLK


def _body(ctx: ExitStack, tc: tile.TileContext, xt, wqkv, wo, cos, sins, masktri, gq, gk, out):
    nc = tc.nc

    const = ctx.enter_context(tc.tile_pool(name="const", bufs=1))
    res = ctx.enter_context(tc.tile_pool(name="res", bufs=1))
    xt_pool = ctx.enter_context(tc.tile_pool(name="xtp", bufs=2))
    sq_pool = ctx.enter_context(tc.tile_pool(name="sqp", bufs=4))
    row_pool = ctx.enter_context(tc.tile_pool(name="row", bufs=6))
    row2_pool = ctx.enter_context(tc.tile_pool(name="row2", bufs=4))
    rope_pool = ctx.enter_context(tc.tile_pool(name="rop", bufs=2))
    exp_pool = ctx.enter_context(tc.tile_pool(name="exq", bufs=6))
    attu_pool = ctx.enter_context(tc.tile_pool(name="attu", bufs=2))
    att_pool = ctx.enter_context(tc.tile_pool(name="attp", bufs=2))
    osb_pool = ctx.enter_context(tc.tile_pool(name="osb", bufs=2))
    # PSUM: 8 banks = sc(3) + attps(1) + sumps(1) + pp(3)
    scp = ctx.enter_context(tc.tile_pool(name="scp", bufs=3, space="PSUM"))
    attps = ctx.enter_context(tc.tile_pool(name="atps", bufs=1, space="PSUM"))
    sumps = ctx.enter_context(tc.tile_pool(name="smps", bufs=1, space="PSUM"))
    pp = ctx.enter_context(tc.tile_pool(name="pp", bufs=3, space="PSUM"))

    # ---- constants / resident weights ----
    ones_bf = const.tile([P, 1], BF, name="ones", tag="ones")
    nc.vector.memset(ones_bf[:], 1.0)
    ones1 = const.tile([1, P], BF, name="ones1", tag="ones1")
    nc.vector.memset(ones1[:], 1.0)
    ident = const.tile([P, P], BF, name="ident", tag="ident")
    make_identity(nc, ident[:])
    gq_t = const.tile([P, 1], F32, name="gq", tag="gq")
    gk_t = const.tile([P, 1], F32, name="gk", tag="gk")
    cos_t = const.tile([P, S], BF, name="cos", tag="cos")
    sins_t = const.tile([P, S], BF, name="sins", tag="sins")
    mask_t = const.tile([P, P], BF, name="mask", tag="mask")
    epsq_t = const.tile([1, 1], F32, name="epsq", tag="epsq")
    nc.vector.memset(epsq_t[:], P * EPS)
    epsk_t = const.tile([1, 1], F32, name="epsk", tag="epsk")
    nc.vector.memset(epsk_t[:], EPS)
    wqkv_sb = [const.tile([P, 768], BF, name=f"wqkv{k}", tag=f"wqkv{k}") for k in range(16)]
    wo_sb = [const.tile([P, D], BF, name=f"wo{h}", tag=f"wo{h}") for h in range(HPC)]

    # resident activations, [dh, token] layouts
    qT = [res.tile([P, S], BF, name=f"qT{h}", tag=f"qT{h}") for h in range(HPC)]
    kT = res.tile([P, S], BF, name="kT", tag="kT")
    vT = res.tile([P, S], BF, name="vT", tag="vT")
    v_kd = res.tile([P, S], BF, name="vkd", tag="vkd")  # [keys, dh] chunks

    xts = [[None] * 16 for _ in range(NBLK)]

    def dma_block(nb):
        for k in range(16):
            t = xt_pool.tile([P, BLK], BF, name=f"xt{k}", tag=f"xt{k}")
            nc.sync.dma_start(t[:], xt[k, :, nb * BLK:(nb + 1) * BLK])
            xts[nb][k] = t

    # preamble DMAs: tiny scales, then wqkv/x-block-0 interleaved so the
    # first matmul can start after ~2 tiles have landed.
    nc.sync.dma_start(gq_t[:], gq[:])
    nc.sync.dma_start(gk_t[:], gk[:])
    for k in range(16):
        nc.sync.dma_start(wqkv_sb[k][:], wqkv[k])
        t = xt_pool.tile([P, BLK], BF, name=f"xt{k}", tag=f"xt{k}")
        nc.sync.dma_start(t[:], xt[k, :, 0:BLK])
        xts[0][k] = t
    nc.sync.dma_start(cos_t[:], cos[:])
    nc.sync.dma_start(sins_t[:], sins[:])
    nc.sync.dma_start(mask_t[:], masktri[:])

    def bcast_row(row_f32):
        """Broadcast a [1,N] f32 row to a [128,N] f32 PSUM tile via two
        accumulating K=1 ones-matmuls over a bf16 hi/lo split (~2^-16 exact)."""
        hi = row2_pool.tile([1, BLK], BF, name="hi", tag="hi")
        lo_f = row_pool.tile([1, BLK], F32, name="lof", tag="row")
        lo = row2_pool.tile([1, BLK], BF, name="lo", tag="lo")
        nc.vector.tensor_copy(hi[:], row_f32[:])
        nc.vector.tensor_tensor(lo_f[:], row_f32[:], hi[:], SUB)
        nc.vector.tensor_copy(lo[:], lo_f[:])
        return hi, lo

    def emit_bcast(hi, lo):
        rsb = pp.tile([P, BLK], F32, name="rsb", tag="pp")
        nc.tensor.matmul(rsb[:], ones1[:], hi[:], start=True, stop=False,
                         skip_group_check=True)
        nc.tensor.matmul(rsb[:], ones1[:], lo[:], start=False, stop=True,
                         skip_group_check=True)
        return rsb

    # ---- phase 1: fused qkv projection + rmsnorm/rope epilogue ----
    # Epilogues are software-pipelined behind the projection matmul stream
    # via a deferred-work queue: stage A (psum evict) runs inline, stage B
    # (sumsq matmul + sqrt + recip) and stage C (rs broadcast + rope) run
    # 1-2 tiles later so their PE instructions never stall the PE.
    from collections import deque
    WORK = deque()

    def pump(n=1):
        for _ in range(n):
            if not WORK:
                return
            nxt = WORK.popleft()()
            if nxt is not None:
                WORK.append(nxt)

    def rope_tile(dst, cols, rsb):
        """dst = (dst*cos + rot(dst)*sin) * rsb, in place; sins has the
        sign of the rotation baked into its first 64 rows."""
        t1 = rope_pool.tile([P, BLK], BF, name="t1", tag="t1")
        t2 = rope_pool.tile([P, BLK], BF, name="t2", tag="t2")
        nc.vector.tensor_copy(t2[0:64, :], dst[64:128, :])
        nc.vector.tensor_copy(t2[64:128, :], dst[0:64, :])
        nc.vector.tensor_tensor(t2[:], t2[:], sins_t[:, cols], MULT)
        nc.vector.tensor_tensor(t1[:], dst[:], cos_t[:, cols], MULT)
        nc.vector.tensor_tensor(t1[:], t1[:], t2[:], ADD)
        nc.vector.tensor_tensor(dst[:], t1[:], rsb[:], MULT)

    def stageA(nb, m, ps):
        cols = slice(nb * BLK, (nb + 1) * BLK)
        if m == 5:  # v: evict now, transpose to [keys, dh] chunks later
            nc.vector.tensor_copy(vT[:, cols], ps[:])

            def stageB_v():
                pst = pp.tile([P, BLK], BF, name="pst", tag="pp")
                for i in range(4):
                    c = nb * 4 + i
                    nc.tensor.transpose(pst[:, i * P:(i + 1) * P],
                                        vT[:, c * P:(c + 1) * P], ident[:])
                nc.scalar.copy(v_kd[:, cols], pst[:])
                return None

            WORK.append(stageB_v)
            return
        if m < 4:
            dst, scale_t, eps_t, escale = qT[m], gq_t, epsq_t, 1.0
        else:
            dst, scale_t, eps_t, escale = kT, gk_t, epsk_t, 1.0 / P
        nc.scalar.activation(dst[:, cols], ps[:], Copy, bias=0.0, scale=scale_t[:])
        sq = sq_pool.tile([P, BLK], BF, name="sq", tag="sq")
        nc.vector.tensor_tensor(sq[:], dst[:, cols], dst[:, cols], MULT)

        def stageB():
            psr = pp.tile([P, BLK], F32, name="psr", tag="pp")
            nc.tensor.matmul(psr[:1, :], ones_bf[:], sq[:], start=True, stop=True,
                             skip_group_check=True)
            row = row_pool.tile([1, BLK], F32, name="row", tag="row")
            # rs_q = 1/sqrt(sumsq+128eps) (absorbs 1/sqrt(dh)); rs_k = rsqrt(var+eps)
            nc.scalar.activation(row[:], psr[:1, :], Sqrt, bias=eps_t[:1, :], scale=escale)
            rrow = row_pool.tile([1, BLK], F32, name="rrow", tag="row")
            nc.vector.reciprocal_approx_fast(rrow[:], row[:])
            hi, lo = bcast_row(rrow)

            def stageC():
                rsb = emit_bcast(hi, lo)
                rope_tile(dst[:, cols], cols, rsb)
                return None

            return stageC

        WORK.append(stageB)

    def proj_block(nb):
        if nb + 1 < NBLK:
            dma_block(nb + 1)
        if nb == 0:
            # k-outer: DMA-paced warmup; uses 6 psum banks across pools
            psms = [scp.tile([P, BLK], F32, name="ps", tag="sc") for _ in range(2)]
            psms.append(attps.tile([P, BLK], F32, name="ps", tag="attps"))
            psms.append(sumps.tile([P, BLK], F32, name="ps", tag="sumps"))
            psms.append(pp.tile([P, BLK], F32, name="ps", tag="pp"))
            psms.append(pp.tile([P, BLK], F32, name="ps", tag="pp"))
            for k in range(16):
                for m in range(6):
                    nc.tensor.matmul(
                        psms[m][:], wqkv_sb[k][:, m * P:(m + 1) * P], xts[0][k][:],
                        start=(k == 0), stop=(k == 15), skip_group_check=True,
                    )
            for m in (5, 0, 1, 2, 3, 4):  # v first: frees its pp slot early
                stageA(0, m, psms[m])
        else:
            for m in range(6):
                ps = pp.tile([P, BLK], F32, name="ps", tag="pp")
                for k in range(16):
                    nc.tensor.matmul(
                        ps[:], wqkv_sb[k][:, m * P:(m + 1) * P], xts[nb][k][:],
                        start=(k == 0), stop=(k == 15), skip_group_check=True,
                    )
                stageA(nb, m, ps)
                pump(2)
        if nb == 1:
            for h in range(HPC):
                nc.sync.dma_start(wo_sb[h][:], wo[h])

    # ---- phase 2: attention (software-pipelined) + Wo per query block ----
    def attn_head(h, qt, atts, inject):
        """Emit scores/exp/AV for (h, qt); the returned closure finishes the
        softmax normalize and is injected into the NEXT head's chunk stream
        so its PE ops never stall the PE. `inject` is the previous head's
        closure."""
        nkc = 4 * qt + 4
        q0 = qt * BLK
        # ps_att/ps_sum are allocated lazily at the first AV matmul, which is
        # emitted AFTER the previous head's norm closure was injected -- so a
        # single buffer per tag is race-free.
        ab = {}

        def scores(kc):
            off = max(0, P * kc - q0)
            ps_s = scp.tile([P, BLK], F32, name="psS", tag="sc")
            nc.tensor.matmul(
                ps_s[:, off:], kT[:, kc * P:(kc + 1) * P], qT[h][:, q0 + off:q0 + BLK],
                start=True, stop=(kc < 4 * qt), skip_group_check=True,
            )
            if kc >= 4 * qt:  # diagonal block: add -30000 upper triangle
                nc.tensor.matmul(
                    ps_s[:, off:off + P], ident[:], mask_t[:],
                    start=False, stop=True, skip_group_check=True,
                )
            ex = exp_pool.tile([P, BLK], BF, name="ex", tag="ex")
            nc.scalar.activation(ex[:, off:], ps_s[:, off:], Exp)
            return kc, off, ex

        def av(kc, off, ex):
            if kc == 0:
                ab["att"] = attps.tile([P, BLK], F32, name="psA", tag="attps")
                ab["sum"] = sumps.tile([P, BLK], F32, name="psB", tag="sumps")
            nc.tensor.matmul(
                ab["att"][:, off:], v_kd[:, kc * P:(kc + 1) * P], ex[:, off:],
                start=(kc == 0), stop=(kc == nkc - 1), skip_group_check=True,
            )
            nc.tensor.matmul(
                ab["sum"][:1, off:], ones_bf[:], ex[:, off:],
                start=(kc == 0), stop=(kc == nkc - 1), skip_group_check=True,
            )

        LAG = 2
        pend = []
        for kc in range(nkc):
            pend.append(scores(kc))
            if kc == 1 and inject is not None:
                inject()
            if len(pend) > LAG:
                av(*pend.pop(0))
        while pend:
            av(*pend.pop(0))

        def norm():
            rrow = row_pool.tile([1, BLK], F32, name="rrow", tag="row")
            nc.vector.reciprocal_approx_fast(rrow[:], ab["sum"][:1, :])
            hi, lo = bcast_row(rrow)
            rsb = emit_bcast(hi, lo)
            att_un = attu_pool.tile([P, BLK], BF, name="attu", tag="attu")
            nc.vector.tensor_copy(att_un[:], ab["att"][:])
            a = att_pool.tile([P, BLK], BF, name=f"att{h}", tag=f"att{h}")
            nc.vector.tensor_tensor(a[:], att_un[:], rsb[:], MULT)
            atts[h] = a

        return norm

    def wo_block(qt, atts, inject):
        q0 = qt * BLK
        for tc4 in range(4):
            osb = osb_pool.tile([P, D], BF, name="osb", tag="osb")
            for et in range(4):
                ps = pp.tile([P, 512], F32, name="pso", tag="pp")
                for h2 in range(HPC):
                    if inject is not None and tc4 == 0 and et == 0 and h2 == 3:
                        inject()
                        inject = None
                    nc.tensor.matmul(
                        ps[:], atts[h2][:, tc4 * P:(tc4 + 1) * P],
                        wo_sb[h2][:, et * 512:(et + 1) * 512],
                        start=(h2 == 0), stop=(h2 == HPC - 1), skip_group_check=True,
                    )
                nc.vector.tensor_copy(osb[:, et * 512:(et + 1) * 512], ps[:])
            nc.sync.dma_start(out[q0 + tc4 * P:q0 + (tc4 + 1) * P, :], osb[:])

    for nb in range(NBLK):
        proj_block(nb)
    while WORK:
        pump()
    for qt in range(NBLK):
        atts = [None] * HPC
        inject = None
        for h in range(HPC):
            inject = attn_head(h, qt, atts, inject)
        wo_block(qt, atts, inject)


_NC_CACHE = None


def build_nc():
    global _NC_CACHE
    if _NC_CACHE is not None:
        return _NC_CACHE
    nc = bacc.Bacc(None, target_bir_lowering=False)
    xt = nc.dram_tensor("xt", [16, P, S], BF, kind="ExternalInput")
    wqkv = nc.dram_tensor("wqkv", [16, P, 768], BF, kind="ExternalInput")
    wo = nc.dram_tensor("wo", [HPC, P, D], BF, kind="ExternalInput")
    cos = nc.dram_tensor("cos", [P, S], BF, kind="ExternalInput")
    sins = nc.dram_tensor("sins", [P, S], BF, kind="ExternalInput")
    masktri = nc.dram_tensor("masktri", [P, P], BF, kind="ExternalInput")
    gq = nc.dram_tensor("gq", [P, 1], F32, kind="ExternalInput")
    gk = nc.dram_tensor("gk", [P, 1], F32, kind="ExternalInput")
    out = nc.dram_tensor("out", [S, D], BF, kind="ExternalOutput")
    with tile.TileContext(nc) as tc:
        with ExitStack() as ctx:
            _body(ctx, tc, xt[:], wqkv[:], wo[:], cos[:], sins[:], masktri[:],
                  gq[:], gk[:], out[:])
    nc.compile()
    _NC_CACHE = nc
    return nc


def _host_tables():
    pos = np.arange(S, dtype=np.float64)
    inv_freq = 1.0 / (ROPE_THETA ** (np.arange(0, DH, 2, dtype=np.float64) / DH))
    ang = pos[:, None] * inv_freq[None, :]  # [S, 64]
    cos_s = np.concatenate([np.cos(ang), np.cos(ang)], axis=-1)  # [S, 128]
    sin_s = np.concatenate([np.sin(ang), np.sin(ang)], axis=-1)
    cos_full = np.ascontiguousarray(cos_s.T).astype(BFNP)  # [128, S]
    sins = sin_s.T.copy()
    sins[0:64] *= -1.0  # rotation sign baked in
    sins = np.ascontiguousarray(sins).astype(BFNP)
    j = np.arange(P)[:, None]
    i = np.arange(P)[None, :]
    masktri = np.where(j <= i, 0.0, -30000.0).astype(BFNP)  # [keys, queries]
    return cos_full, sins, masktri


def kernel(qkv, Wq, Wk, Wv, Wo, q_gamma, k_gamma):
    qkv = np.asarray(qkv, dtype=np.float32)
    Wq = np.asarray(Wq, dtype=np.float32)
    Wk = np.asarray(Wk, dtype=np.float32)
    Wv = np.asarray(Wv, dtype=np.float32)
    Wo = np.asarray(Wo, dtype=np.float32)
    q_gamma = np.asarray(q_gamma, dtype=np.float32)
    k_gamma = np.asarray(k_gamma, dtype=np.float32)

    nc = build_nc()
    cos_full, sins, masktri = _host_tables()
    gq = np.ascontiguousarray(q_gamma.reshape(P, 1))
    gk = np.ascontiguousarray(k_gamma.reshape(P, 1))
    xts = [np.ascontiguousarray(qkv[b].T).astype(BFNP).reshape(16, P, S) for b in range(B)]

    in_maps = []
    for c in range(NCORES):
        b, g = c // 4, c % 4
        wq_c = Wq[4 * g * DH:(4 * g + 4) * DH, :]  # [512, D]
        wk_c = Wk[g * DH:(g + 1) * DH, :]  # [128, D]
        wv_c = Wv[g * DH:(g + 1) * DH, :]
        wqkv_c = np.concatenate([wq_c, wk_c, wv_c], axis=0).T  # [D, 768]
        wqkv_c = np.ascontiguousarray(wqkv_c).astype(BFNP).reshape(16, P, 768)
        wo_c = np.stack(
            [np.ascontiguousarray(Wo[:, (4 * g + h) * DH:(4 * g + h + 1) * DH].T)
             for h in range(HPC)]
        ).astype(BFNP)  # [4, 128, D]
        in_maps.append({
            "xt": xts[b], "wqkv": wqkv_c, "wo": wo_c,
            "cos": cos_full, "sins": sins, "masktri": masktri,
            "gq": gq, "gk": gk,
        })

    res = run_bass_kernel_spmd(nc, in_maps, core_ids=list(range(NCORES)))
    full = np.empty((B, S, D), np.float32)
    for b in range(B):
        acc = res.results[4 * b]["out"].astype(np.float32)
        for g in range(1, 4):
            acc += res.results[4 * b + g]["out"].astype(np.float32)
        full[b] = acc
    return full


# revision 20
# speedup vs baseline: 1.3543x; 1.1268x over previous
"""Causal GQA multi-head attention (RMSNorm-QK + RoPE) on 8 Trainium2 cores.

Sharding: (batch, kv-group). Core c owns batch c//4 and GQA group c%4,
i.e. 4 q heads + 1 kv head for one batch of 2048 tokens. This splits the
total work exactly 8 ways with zero duplicated projection flops (the old
head-sharding computed each kv head twice and projected both batches on
every core). Each core emits a partial [S, D] output (row-sharded Wo);
the host sums the 4 partials per batch.

Per-core structure (all matmuls bf16, K=M=128, N<=512):
  - proj: x[D, S] @ wqkv -> qT (4 heads), kT, vT in [dh, token] layout.
    First block is emitted k-outer so the PE starts after ~2 DMA tiles.
  - rmsnorm: sumsq via ones-matmul; rsqrt as exp(-0.5*ln(x)) so the
    whole kernel uses ONE activation table set (natural_log_exp);
    q-side rs (absorbs 1/sqrt(dh)) and k-side rs are folded into the
    qT/kT tiles themselves during the RoPE epilogue, so attention exp
    needs no per-partition scale.
  - attention: scoresT blocks [keys, queries]; causal mask folded into
    the scores psum via a tiny [128,128] identity-matmul add of -30000
    on the diagonal block only; exp WITHOUT max-subtraction; rowsums
    via accumulated ones-matmul; the inner loop is software-pipelined
    (scores run LAG=2 chunks ahead of AV) so the PE never waits on the
    scalar engine's exp.
  - Wo: row-sharded partial, interleaved per query-block.
"""

import sys

sys.path.insert(0, "/opt/trn_rl_repo")

from contextlib import ExitStack

import ml_dtypes
import numpy as np

import concourse.bass as bass
import concourse.tile as tile
from concourse import bacc, mybir
from concourse.bass_utils import run_bass_kernel_spmd
from concourse.masks import make_identity

B, S, D = 2, 2048, 2048
H, HKV, DH = 16, 4, 128
P = 128
NCORES = 8
HPC = 4  # q heads per core
EPS = 1e-6
ROPE_THETA = 10000.0
BF = mybir.dt.bfloat16
F32 = mybir.dt.float32
BFNP = ml_dtypes.bfloat16

Copy = mybir.ActivationFunctionType.Copy
Exp = mybir.ActivationFunctionType.Exp
Sqrt = mybir.ActivationFunctionType.Sqrt
MULT = mybir.AluOpType.mult
ADD = mybir.AluOpType.add
SUB = mybir.AluOpType.subtract

NBLK = 4  # 512-token blocks
BLK = S // NBLK


def _body(ctx: ExitStack, tc: tile.TileContext, xt, wqkv, wo, cos, sins, masktri, gq, gk, out):
    nc = tc.nc

    const = ctx.enter_context(tc.tile_pool(name="const", bufs=1))
    res = ctx.enter_context(tc.tile_pool(name="res", bufs=1))
    xt_pool = ctx.enter_context(tc.tile_pool(name="xtp", bufs=2))
    sq_pool = ctx.enter_context(tc.tile_pool(name="sqp", bufs=4))
    row_pool = ctx.enter_context(tc.tile_pool(name="row", bufs=6))
    row2_pool = ctx.enter_context(tc.tile_pool(name="row2", bufs=4))
    rope_pool = ctx.enter_context(tc.tile_pool(name="rop", bufs=2))
    exp_pool = ctx.enter_context(tc.tile_pool(name="exq", bufs=6))
    attu_pool = ctx.enter_context(tc.tile_pool(name="attu", bufs=2))
    att_pool = ctx.enter_context(tc.tile_pool(name="attp", bufs=2))
    osb_pool = ctx.enter_context(tc.tile_pool(name="osb", bufs=2))
    # PSUM: 8 banks = sc(3) + attps(1) + sumps(1) + pp(3)
    scp = ctx.enter_context(tc.tile_pool(name="scp", bufs=3, space="PSUM"))
    attps = ctx.enter_context(tc.tile_pool(name="atps", bufs=1, space="PSUM"))
    sumps = ctx.enter_context(tc.tile_pool(name="smps", bufs=1, space="PSUM"))
    pp = ctx.enter_context(tc.tile_pool(name="pp", bufs=3, space="PSUM"))

    # ---- constants / resident weights ----
    ones_sq = const.tile([P, P], BF, name="ones", tag="ones")
    nc.vector.memset(ones_sq[:], 1.0)
    ones1 = const.tile([1, P], BF, name="ones1", tag="ones1")
    nc.vector.memset(ones1[:], 1.0)
    ident = const.tile([P, P], BF, name="ident", tag="ident")
    make_identity(nc, ident[:])
    gq_t = const.tile([P, 1], F32, name="gq", tag="gq")
    gk_t = const.tile([P, 1], F32, name="gk", tag="gk")
    cos_t = const.tile([P, S], BF, name="cos", tag="cos")
    sins_t = const.tile([P, S], BF, name="sins", tag="sins")
    mask_t = const.tile([P, P], BF, name="mask", tag="mask")
    epsq_t = const.tile([1, 1], F32, name="epsq", tag="epsq")
    nc.vector.memset(epsq_t[:], P * EPS)
    epsk_t = const.tile([1, 1], F32, name="epsk", tag="epsk")
    nc.vector.memset(epsk_t[:], EPS)
    wqkv_sb = [const.tile([P, 768], BF, name=f"wqkv{k}", tag=f"wqkv{k}") for k in range(16)]
    wo_sb = [const.tile([P, D], BF, name=f"wo{h}", tag=f"wo{h}") for h in range(HPC)]

    # resident activations, [dh, token] layouts
    qT = [res.tile([P, S], BF, name=f"qT{h}", tag=f"qT{h}") for h in range(HPC)]
    kT = res.tile([P, S], BF, name="kT", tag="kT")
    vT = res.tile([P, S], BF, name="vT", tag="vT")
    v_kd = res.tile([P, S], BF, name="vkd", tag="vkd")  # [keys, dh] chunks

    xts = [[None] * 16 for _ in range(NBLK)]

    def dma_block(nb):
        for k in range(16):
            t = xt_pool.tile([P, BLK], BF, name=f"xt{k}", tag=f"xt{k}")
            nc.sync.dma_start(t[:], xt[k, :, nb * BLK:(nb + 1) * BLK])
            xts[nb][k] = t

    # preamble DMAs: tiny scales, then wqkv/x-block-0 interleaved so the
    # first matmul can start after ~2 tiles have landed.
    for k in range(16):
        nc.sync.dma_start(wqkv_sb[k][:], wqkv[k])
        t = xt_pool.tile([P, BLK], BF, name=f"xt{k}", tag=f"xt{k}")
        nc.sync.dma_start(t[:], xt[k, :, 0:BLK])
        xts[0][k] = t
        if k == 3:
            nc.sync.dma_start(gq_t[:], gq[:])
            nc.sync.dma_start(gk_t[:], gk[:])
    nc.sync.dma_start(cos_t[:], cos[:])
    nc.sync.dma_start(sins_t[:], sins[:])
    nc.sync.dma_start(mask_t[:], masktri[:])

    def bcast_row(row_f32):
        """Round a [1,N] f32 row to bf16 for the K=1 broadcast matmul
        (~0.1% rms rounding, well within tolerance)."""
        hi = row2_pool.tile([1, BLK], BF, name="hi", tag="hi")
        nc.vector.tensor_copy(hi[:], row_f32[:])
        return hi

    def emit_bcast(hi):
        rsb = pp.tile([P, BLK], F32, name="rsb", tag="pp")
        nc.tensor.matmul(rsb[:], ones1[:], hi[:], start=True, stop=True,
                         skip_group_check=True)
        return rsb

    # ---- phase 1: fused qkv projection + rmsnorm/rope epilogue ----
    # Epilogues are software-pipelined behind the projection matmul stream
    # via a deferred-work queue: stage A (psum evict) runs inline, stage B
    # (sumsq matmul + sqrt + recip) and stage C (rs broadcast + rope) run
    # 1-2 tiles later so their PE instructions never stall the PE.
    from collections import deque
    WORK = deque()

    def pump(n=1):
        for _ in range(n):
            if not WORK:
                return
            nxt = WORK.popleft()()
            if nxt is not None:
                WORK.append(nxt)

    def rope_tile(dst, cols, rsb):
        """dst = (dst*cos + rot(dst)*sin) * rsb, in place; sins has the
        sign of the rotation baked into its first 64 rows."""
        t1 = rope_pool.tile([P, BLK], BF, name="t1", tag="t1")
        t2 = rope_pool.tile([P, BLK], BF, name="t2", tag="t2")
        nc.vector.tensor_copy(t2[0:64, :], dst[64:128, :])
        nc.vector.tensor_copy(t2[64:128, :], dst[0:64, :])
        nc.vector.tensor_tensor(t2[:], t2[:], sins_t[:, cols], MULT)
        nc.vector.tensor_tensor(t1[:], dst[:], cos_t[:, cols], MULT)
        nc.vector.tensor_tensor(t1[:], t1[:], t2[:], ADD)
        nc.vector.tensor_tensor(dst[:], t1[:], rsb[:], MULT)

    def stageA(nb, m, ps):
        cols = slice(nb * BLK, (nb + 1) * BLK)
        if m == 5:  # v: evict now, transpose to [keys, dh] chunks later
            nc.vector.tensor_copy(vT[:, cols], ps[:])

            def stageB_v():
                pst = pp.tile([P, BLK], BF, name="pst", tag="pp")
                for i in range(4):
                    c = nb * 4 + i
                    nc.tensor.transpose(pst[:, i * P:(i + 1) * P],
                                        vT[:, c * P:(c + 1) * P], ident[:])
                nc.scalar.copy(v_kd[:, cols], pst[:])
                return None

            WORK.append(stageB_v)
            return
        if m < 4:
            dst, scale_t, eps_t, escale = qT[m], gq_t, epsq_t, 1.0
        else:
            dst, scale_t, eps_t, escale = kT, gk_t, epsk_t, 1.0 / P
        nc.scalar.activation(dst[:, cols], ps[:], Copy, bias=0.0, scale=scale_t[:])
        sq = sq_pool.tile([P, BLK], BF, name="sq", tag="sq")
        nc.vector.tensor_tensor(sq[:], dst[:, cols], dst[:, cols], MULT)

        def stageB():
            psr = pp.tile([P, BLK], F32, name="psr", tag="pp")
            nc.tensor.matmul(psr[:], ones_sq[:], sq[:], start=True, stop=True,
                             skip_group_check=True)
            row = row_pool.tile([1, BLK], F32, name="row", tag="row")
            # rs_q = 1/sqrt(sumsq+128eps) (absorbs 1/sqrt(dh)); rs_k = rsqrt(var+eps)
            nc.scalar.activation(row[:], psr[:1, :], Sqrt, bias=eps_t[:1, :], scale=escale)
            rrow = row_pool.tile([1, BLK], F32, name="rrow", tag="row")
            nc.vector.reciprocal_approx_fast(rrow[:], row[:])
            hi = bcast_row(rrow)

            def stageC():
                rsb = emit_bcast(hi)
                rope_tile(dst[:, cols], cols, rsb)
                return None

            return stageC

        WORK.append(stageB)

    def proj_block(nb):
        if nb + 1 < NBLK:
            dma_block(nb + 1)
        if nb == 0:
            # k-outer: DMA-paced warmup; uses 6 psum banks across pools
            psms = [scp.tile([P, BLK], F32, name="ps", tag="sc") for _ in range(2)]
            psms.append(attps.tile([P, BLK], F32, name="ps", tag="attps"))
            psms.append(sumps.tile([P, BLK], F32, name="ps", tag="sumps"))
            psms.append(pp.tile([P, BLK], F32, name="ps", tag="pp"))
            psms.append(pp.tile([P, BLK], F32, name="ps", tag="pp"))
            for k in range(16):
                for m in range(6):
                    nc.tensor.matmul(
                        psms[m][:], wqkv_sb[k][:, m * P:(m + 1) * P], xts[0][k][:],
                        start=(k == 0), stop=(k == 15), skip_group_check=True,
                    )
            for m in (5, 0, 1, 2, 3, 4):  # v first: frees its pp slot early
                stageA(0, m, psms[m])
        else:
            for m in range(6):
                ps = pp.tile([P, BLK], F32, name="ps", tag="pp")
                for k in range(16):
                    nc.tensor.matmul(
                        ps[:], wqkv_sb[k][:, m * P:(m + 1) * P], xts[nb][k][:],
                        start=(k == 0), stop=(k == 15), skip_group_check=True,
                    )
                stageA(nb, m, ps)
                pump(2)
        if nb == 1:
            for h in range(HPC):
                nc.sync.dma_start(wo_sb[h][:], wo[h])

    # ---- phase 2: attention (software-pipelined) + Wo per query block ----
    def attn_head(h, qt, atts, inject):
        """Emit scores/exp/AV for (h, qt); the returned closure finishes the
        softmax normalize and is injected into the NEXT head's chunk stream
        so its PE ops never stall the PE. `inject` is the previous head's
        closure."""
        nkc = 4 * qt + 4
        q0 = qt * BLK
        # ps_att/ps_sum are allocated lazily at the first AV matmul, which is
        # emitted AFTER the previous head's norm closure was injected -- so a
        # single buffer per tag is race-free.
        ab = {}

        def scores(kc):
            off = max(0, P * kc - q0)
            ps_s = scp.tile([P, BLK], F32, name="psS", tag="sc")
            nc.tensor.matmul(
                ps_s[:, off:], kT[:, kc * P:(kc + 1) * P], qT[h][:, q0 + off:q0 + BLK],
                start=True, stop=(kc < 4 * qt), skip_group_check=True,
            )
            if kc >= 4 * qt:  # diagonal block: add -30000 upper triangle
                nc.tensor.matmul(
                    ps_s[:, off:off + P], ident[:], mask_t[:],
                    start=False, stop=True, skip_group_check=True,
                )
            ex = exp_pool.tile([P, BLK], BF, name="ex", tag="ex")
            nc.scalar.activation(ex[:, off:], ps_s[:, off:], Exp)
            return kc, off, ex

        def av(kc, off, ex):
            if kc == 0:
                ab["att"] = attps.tile([P, BLK], F32, name="psA", tag="attps")
                ab["sum"] = sumps.tile([P, BLK], F32, name="psB", tag="sumps")
            nc.tensor.matmul(
                ab["att"][:, off:], v_kd[:, kc * P:(kc + 1) * P], ex[:, off:],
                start=(kc == 0), stop=(kc == nkc - 1), skip_group_check=True,
            )
            nc.tensor.matmul(
                ab["sum"][:, off:], ones_sq[:], ex[:, off:],
                start=(kc == 0), stop=(kc == nkc - 1), skip_group_check=True,
            )

        LAG = 2
        pend = []
        for kc in range(nkc):
            pend.append(scores(kc))
            if kc == 1 and inject is not None:
                inject()
            if len(pend) > LAG:
                av(*pend.pop(0))
        while pend:
            av(*pend.pop(0))

        def norm():
            rrow = row_pool.tile([1, BLK], F32, name="rrow", tag="row")
            nc.vector.reciprocal_approx_fast(rrow[:], ab["sum"][:1, :])
            rsb = emit_bcast(bcast_row(rrow))
            att_un = attu_pool.tile([P, BLK], BF, name="attu", tag="attu")
            nc.vector.tensor_copy(att_un[:], ab["att"][:])
            a = att_pool.tile([P, BLK], BF, name=f"att{h}", tag=f"att{h}")
            nc.vector.tensor_tensor(a[:], att_un[:], rsb[:], MULT)
            atts[h] = a

        return norm

    def wo_block(qt, atts, inject):
        q0 = qt * BLK
        for tc4 in range(4):
            osb = osb_pool.tile([P, D], BF, name="osb", tag="osb")
            for et in range(4):
                ps = pp.tile([P, 512], F32, name="pso", tag="pp")
                for h2 in range(HPC):
                    if inject is not None and tc4 == 0 and et == 0 and h2 == 3:
                        inject()
                        inject = None
                    nc.tensor.matmul(
                        ps[:], atts[h2][:, tc4 * P:(tc4 + 1) * P],
                        wo_sb[h2][:, et * 512:(et + 1) * 512],
                        start=(h2 == 0), stop=(h2 == HPC - 1), skip_group_check=True,
                    )
                nc.vector.tensor_copy(osb[:, et * 512:(et + 1) * 512], ps[:])
            nc.sync.dma_start(out[q0 + tc4 * P:q0 + (tc4 + 1) * P, :], osb[:])

    for nb in range(NBLK):
        proj_block(nb)
    while WORK:
        pump()
    for qt in range(NBLK):
        atts = [None] * HPC
        inject = None
        for h in range(HPC):
            inject = attn_head(h, qt, atts, inject)
        wo_block(qt, atts, inject)


_NC_CACHE = None


def build_nc():
    global _NC_CACHE
    if _NC_CACHE is not None:
        return _NC_CACHE
    nc = bacc.Bacc(None, target_bir_lowering=False)
    xt = nc.dram_tensor("xt", [16, P, S], BF, kind="ExternalInput")
    wqkv = nc.dram_tensor("wqkv", [16, P, 768], BF, kind="ExternalInput")
    wo = nc.dram_tensor("wo", [HPC, P, D], BF, kind="ExternalInput")
    cos = nc.dram_tensor("cos", [P, S], BF, kind="ExternalInput")
    sins = nc.dram_tensor("sins", [P, S], BF, kind="ExternalInput")
    masktri = nc.dram_tensor("masktri", [P, P], BF, kind="ExternalInput")
    gq = nc.dram_tensor("gq", [P, 1], F32, kind="ExternalInput")
    gk = nc.dram_tensor("gk", [P, 1], F32, kind="ExternalInput")
    out = nc.dram_tensor("out", [S, D], BF, kind="ExternalOutput")
    with tile.TileContext(nc) as tc:
        with ExitStack() as ctx:
            _body(ctx, tc, xt[:], wqkv[:], wo[:], cos[:], sins[:], masktri[:],
                  gq[:], gk[:], out[:])
    nc.compile()
    _NC_CACHE = nc
    return nc


def _host_tables():
    pos = np.arange(S, dtype=np.float64)
    inv_freq = 1.0 / (ROPE_THETA ** (np.arange(0, DH, 2, dtype=np.float64) / DH))
    ang = pos[:, None] * inv_freq[None, :]  # [S, 64]
    cos_s = np.concatenate([np.cos(ang), np.cos(ang)], axis=-1)  # [S, 128]
    sin_s = np.concatenate([np.sin(ang), np.sin(ang)], axis=-1)
    cos_full = np.ascontiguousarray(cos_s.T).astype(BFNP)  # [128, S]
    sins = sin_s.T.copy()
    sins[0:64] *= -1.0  # rotation sign baked in
    sins = np.ascontiguousarray(sins).astype(BFNP)
    j = np.arange(P)[:, None]
    i = np.arange(P)[None, :]
    masktri = np.where(j <= i, 0.0, -30000.0).astype(BFNP)  # [keys, queries]
    return cos_full, sins, masktri


def kernel(qkv, Wq, Wk, Wv, Wo, q_gamma, k_gamma):
    qkv = np.asarray(qkv, dtype=np.float32)
    Wq = np.asarray(Wq, dtype=np.float32)
    Wk = np.asarray(Wk, dtype=np.float32)
    Wv = np.asarray(Wv, dtype=np.float32)
    Wo = np.asarray(Wo, dtype=np.float32)
    q_gamma = np.asarray(q_gamma, dtype=np.float32)
    k_gamma = np.asarray(k_gamma, dtype=np.float32)

    nc = build_nc()
    cos_full, sins, masktri = _host_tables()
    gq = np.ascontiguousarray(q_gamma.reshape(P, 1))
    gk = np.ascontiguousarray(k_gamma.reshape(P, 1))
    xts = [np.ascontiguousarray(qkv[b].T).astype(BFNP).reshape(16, P, S) for b in range(B)]

    in_maps = []
    for c in range(NCORES):
        b, g = c // 4, c % 4
        wq_c = Wq[4 * g * DH:(4 * g + 4) * DH, :]  # [512, D]
        wk_c = Wk[g * DH:(g + 1) * DH, :]  # [128, D]
        wv_c = Wv[g * DH:(g + 1) * DH, :]
        wqkv_c = np.concatenate([wq_c, wk_c, wv_c], axis=0).T  # [D, 768]
        wqkv_c = np.ascontiguousarray(wqkv_c).astype(BFNP).reshape(16, P, 768)
        wo_c = np.stack(
            [np.ascontiguousarray(Wo[:, (4 * g + h) * DH:(4 * g + h + 1) * DH].T)
             for h in range(HPC)]
        ).astype(BFNP)  # [4, 128, D]
        in_maps.append({
            "xt": xts[b], "wqkv": wqkv_c, "wo": wo_c,
            "cos": cos_full, "sins": sins, "masktri": masktri,
            "gq": gq, "gk": gk,
        })

    res = run_bass_kernel_spmd(nc, in_maps, core_ids=list(range(NCORES)))
    full = np.empty((B, S, D), np.float32)
    for b in range(B):
        acc = res.results[4 * b]["out"].astype(np.float32)
        for g in range(1, 4):
            acc += res.results[4 * b + g]["out"].astype(np.float32)
        full[b] = acc
    return full
